# revision 17
# baseline (speedup 1.0000x reference)
"""Trainium2 Bass kernel for nn_AppearanceLoss (keypoint patch CNN MSE).

Host: crops 33x33 patches at keypoint locations (data-dependent indices),
builds the conv1 im2col directly (9 shifted copies + ones row for the bias),
shards 256 keypoints across 8 NeuronCores.
Device: conv1 = single K=109 matmul per 496-pixel chunk (im2col weights);
conv2 = offset-accumulated K=64 matmuls over patch pairs, two concurrent
row-tiles, N=450 (two quads per matmul); GAP via DVE reduce; linear on
feature diffs (bias cancels); Square+accum partial sums.
Host: sums 8 per-core partials into the scalar MSE.
"""

import sys

sys.path.insert(0, "/opt/trn_rl_repo")

from contextlib import ExitStack

import ml_dtypes
import numpy as np

import concourse.bass as bass  # noqa: F401
import concourse.tile as tile
from concourse import bacc, bass_utils, mybir

SIGMA = 16
PATCH = 33  # 2*SIGMA+1
HOUT = 31  # conv1 valid output: 33-3+1
COUT = 15  # conv2 stride-2 valid output: (31-3)//2+1
B, K, H = 4, 64, 256
NCORES = 8
NKP = B * K  # 256 keypoints total
KPC = NKP // NCORES  # 32 keypoints per core
NPATCH = KPC * B  # 128 patches per core per set
NQ = NPATCH // 4  # 32 quads per set
NQT = 2 * NQ  # 64 quads total per core
KIM = 109  # conv1 im2col contraction: 9 offsets * 12 (patch,chan) + ones row
BF16 = mybir.dt.bfloat16
F32 = mybir.dt.float32
NPBF16 = ml_dtypes.bfloat16

_CACHE: dict = {}


def _build_graph():
    nc = bacc.Bacc(
        "TRN2",
        target_bir_lowering=False,
        debug=False,
        enable_asserts=False,
        num_devices=NCORES,
    )
    # conv1 im2col input: row o*12+3j+c = offset o, patch-in-quad j, chan c;
    # row 108 = const 1.0 (bias); col = quad (0..63), then 31x31 out pixels
    xim_d = nc.dram_tensor(
        "xim", [KIM, NQT, HOUT, HOUT], BF16, kind="ExternalInput"
    ).ap()
    w1_d = nc.dram_tensor("w1", [KIM, 128], BF16, kind="ExternalInput").ap()
    w2_d = nc.dram_tensor("w2", [128, 9, 128], BF16, kind="ExternalInput").ap()
    b2_d = nc.dram_tensor("b2", [128, 1], F32, kind="ExternalInput").ap()
    wl_d = nc.dram_tensor("wl", [128, 128], BF16, kind="ExternalInput").ap()
    out_d = nc.dram_tensor("out", [128, 2], F32, kind="ExternalOutput").ap()

    RELU = mybir.ActivationFunctionType.Relu
    SQUARE = mybir.ActivationFunctionType.Square

    with ExitStack() as ctx:
        tc = ctx.enter_context(tile.TileContext(nc))
        const = ctx.enter_context(tc.tile_pool(name="const", bufs=1))
        xpool = ctx.enter_context(tc.tile_pool(name="x", bufs=2))
        hpool = ctx.enter_context(tc.tile_pool(name="h", bufs=1))
        gpool = ctx.enter_context(tc.tile_pool(name="g", bufs=1))
        spool = ctx.enter_context(tc.tile_pool(name="scr", bufs=4))
        pp1 = ctx.enter_context(tc.tile_pool(name="pp1", bufs=3, space="PSUM"))
        pp2 = ctx.enter_context(tc.tile_pool(name="pp2", bufs=5, space="PSUM"))

        w1_t = const.tile([KIM, 128], BF16)
        nc.sync.dma_start(w1_t[:], w1_d)
        w2_t = const.tile([128, 9, 128], BF16)
        nc.sync.dma_start(w2_t[:], w2_d)
        b2_t = const.tile([128, 1], F32)
        nc.sync.dma_start(b2_t[:], b2_d)
        wl_t = const.tile([128, 128], BF16)
        nc.sync.dma_start(wl_t[:], wl_d)

        # gap col 2q+jj; partition 64a+m = patch (q, 2*jj+a) channel m
        gap = gpool.tile([128, NQT * 2], F32)
        res = gpool.tile([128, 2], F32)

        for s in range(2):  # 0=ground, 1=satellite
            # conv1 out: partition 32j+m
            h1 = hpool.tile([128, NQ, HOUT, HOUT], BF16, tag="h1")
            for half in range(2):  # 16 quads per DMA slice
                xim = xpool.tile([KIM, NQ // 2, HOUT, HOUT], BF16, tag="xim")
                q0 = s * NQ + half * (NQ // 2)
                nc.sync.dma_start(xim[:], xim_d[:, q0 : q0 + NQ // 2])
                for qq in range(NQ // 2):
                    q = half * (NQ // 2) + qq  # quad index within set
                    for ci, (r0, nr) in enumerate(((0, 16), (16, 15))):
                        ps1 = pp1.tile([128, 16, HOUT], F32, tag="ps1")
                        nc.tensor.matmul(
                            ps1[:, :nr, :],
                            w1_t[:],
                            xim[:, qq, r0 : r0 + nr, :],
                            start=True,
                            stop=True,
                        )
                        # relu eviction (bias folded into im2col ones row);
                        # alternate engines to split the eviction load
                        if (qq + ci) % 2 == 0:
                            nc.scalar.activation(
                                h1[:, q, r0 : r0 + nr, :], ps1[:, :nr, :], RELU
                            )
                        else:
                            nc.vector.tensor_scalar_max(
                                h1[:, q, r0 : r0 + nr, :], ps1[:, :nr, :], 0.0
                            )
            # conv2: 2-quad-wide matmuls (N=450), pairs in concurrent row-tiles
            for G in range(NQ // 2):
                for jj in range(2):
                    p0 = 64 * jj
                    ps2 = pp2.tile(
                        [128, 2, COUT * COUT], F32, tag="ps2", name=f"ps2_{jj}"
                    )
                    for o in range(9):
                        dy, dx = o // 3, o % 3
                        nc.tensor.matmul(
                            ps2[:],
                            w2_t[p0 : p0 + 64, o, :],
                            h1[
                                p0 : p0 + 64,
                                2 * G : 2 * G + 2,
                                dy : dy + 29 : 2,
                                dx : dx + 29 : 2,
                            ],
                            start=(o == 0),
                            stop=(o == 8),
                            tile_position=(p0, 0),
                        )
                    scr = spool.tile([128, 2, COUT * COUT], F32, tag="scr")
                    nc.scalar.activation(
                        scr[:],
                        ps2[:],
                        RELU,
                        bias=b2_t[:],
                        scale=1.0 / (COUT * COUT),
                    )
                    # GAP: per-quad sums -> gap cols 2*(s*NQ+2G+k)+jj, k=0,1
                    c0 = 2 * (s * NQ + 2 * G) + jj
                    nc.vector.tensor_reduce(
                        gap[:, c0 : c0 + 3 : 2],
                        scr[:],
                        axis=mybir.AxisListType.X,
                        op=mybir.AluOpType.add,
                    )

        # linear on feature diffs (linear bias cancels), squared sums
        dg = spool.tile([128, NQ * 2], F32, tag="dg")
        nc.vector.tensor_sub(dg[:], gap[:, 0 : NQ * 2], gap[:, NQ * 2 : NQT * 2])
        dgb = spool.tile([128, NQ * 2], BF16, tag="dgb")
        nc.vector.tensor_copy(dgb[:], dg[:])
        for jj in range(2):
            p0 = 64 * jj
            ps3 = pp2.tile([128, NQ * 2], F32, tag="ps2", name=f"ps3_{jj}")
            nc.tensor.matmul(
                ps3[:],
                wl_t[p0 : p0 + 64, :],
                dgb[p0 : p0 + 64, :],
                start=True,
                stop=True,
                tile_position=(p0, 0),
            )
            scr3 = spool.tile([128, NQ * 2], F32, tag="scr3", name=f"scr3_{jj}")
            nc.scalar.activation(
                scr3[:], ps3[:], SQUARE, accum_out=res[:, jj : jj + 1]
            )
        nc.sync.dma_start(out_d, res[:])

    nc.compile()
    return nc


def _prep_weights(w1, b1, w2, b2, wl):
    w1im = np.zeros((KIM, 128), np.float32)
    for j in range(4):
        for c in range(3):
            for o in range(9):
                dy, dx = o // 3, o % 3
                w1im[12 * o + 3 * j + c, 32 * j : 32 * j + 32] = w1[:, c, dy, dx]
    w1im[108, :] = np.tile(b1, 4)
    w2blk = np.zeros((128, 9, 128), np.float32)
    for jj in range(2):
        for j in range(2):
            for c in range(32):
                for o in range(9):
                    dy, dx = o // 3, o % 3
                    w2blk[64 * jj + 32 * j + c, o, 64 * j : 64 * j + 64] = w2[
                        :, c, dy, dx
                    ]
    b2q = np.tile(b2 / (COUT * COUT), 2)[:, None].astype(np.float32)
    wlrep = np.zeros((128, 128), np.float32)
    wlrep[0:64] = wl.T
    wlrep[64:128] = wl.T
    return (
        w1im.astype(NPBF16),
        w2blk.astype(NPBF16),
        np.ascontiguousarray(b2q),
        wlrep.astype(NPBF16),
    )


def _crop_all(images, kps):
    # images [B,3,H,W] f32; kps [NKP,2] normalized -> patches [NKP,B,3,P,P]
    hw = images.shape[-1]
    px = kps.astype(np.float32) * np.float32(hw)
    starts = np.clip(np.floor(px).astype(np.int32) - SIGMA, 0, hw - PATCH)
    out = np.empty((kps.shape[0], images.shape[0], 3, PATCH, PATCH), np.float32)
    for n in range(kps.shape[0]):
        x, y = int(starts[n, 0]), int(starts[n, 1])
        out[n] = images[:, :, y : y + PATCH, x : x + PATCH]
    return out


def _im2col(pat):
    # [128,3,33,33] -> [KIM, 32, 31, 31]: row 12o+3j+c = patch data shifted
    # by offset o's (dy,dx); row 108 = 1.0
    pat4 = pat.reshape(NQ, 4, 3, PATCH, PATCH)
    out = np.empty((KIM, NQ, HOUT, HOUT), np.float32)
    for o in range(9):
        dy, dx = o // 3, o % 3
        sh = pat4[:, :, :, dy : dy + HOUT, dx : dx + HOUT]
        out[12 * o : 12 * o + 12] = sh.transpose(1, 2, 0, 3, 4).reshape(
            12, NQ, HOUT, HOUT
        )
    out[108] = 1.0
    return out


def _make_in_maps(np_inputs):
    images_ground = np.asarray(np_inputs["images_ground"], np.float32)
    images_satellite = np.asarray(np_inputs["images_satellite"], np.float32)
    kg = np.asarray(np_inputs["keypoints_ground"], np.float32).reshape(-1, 2)
    ks = np.asarray(np_inputs["keypoints_satellite"], np.float32).reshape(-1, 2)
    w1 = np.asarray(np_inputs["w1"], np.float32)
    b1 = np.asarray(np_inputs["b1"], np.float32)
    w2 = np.asarray(np_inputs["w2"], np.float32)
    b2 = np.asarray(np_inputs["b2"], np.float32)
    wl = np.asarray(np_inputs["wl"], np.float32)

    pg = _crop_all(images_ground, kg)  # [256,4,3,33,33]
    ps = _crop_all(images_satellite, ks)
    w1im, w2blk, b2q, wlrep = _prep_weights(w1, b1, w2, b2, wl)

    in_maps = []
    for i in range(NCORES):
        sl = slice(i * KPC, (i + 1) * KPC)
        patg = pg[sl].reshape(NPATCH, 3, PATCH, PATCH)
        pats = ps[sl].reshape(NPATCH, 3, PATCH, PATCH)
        xim = np.concatenate([_im2col(patg), _im2col(pats)], axis=1).astype(NPBF16)
        in_maps.append(dict(xim=xim, w1=w1im, w2=w2blk, b2=b2q, wl=wlrep))
    return in_maps


def kernel(**inputs):
    in_maps = _make_in_maps(inputs)

    if "nc" not in _CACHE:
        _CACHE["nc"] = _build_graph()
    nc = _CACHE["nc"]

    results = bass_utils.run_bass_kernel_spmd(
        nc, in_maps, core_ids=list(range(NCORES))
    )
    total = np.float64(0.0)
    for r in results.results:
        total += np.asarray(r["out"], np.float64).sum()
    mse = total / (NKP * B * 128)
    return np.asarray(mse, np.float32)


if __name__ == "__main__":
    rng = np.random.default_rng(0)
    ins = dict(
        images_ground=rng.standard_normal((B, 3, H, H)).astype(np.float32),
        images_satellite=rng.standard_normal((B, 3, H, H)).astype(np.float32),
        keypoints_ground=(0.2 + 0.6 * rng.random((B, K, 2))).astype(np.float32),
        keypoints_satellite=(0.2 + 0.6 * rng.random((B, K, 2))).astype(np.float32),
        w1=(rng.standard_normal((32, 3, 3, 3)) * 0.1).astype(np.float32),
        b1=np.zeros(32, np.float32),
        w2=(rng.standard_normal((64, 32, 3, 3)) * 0.05).astype(np.float32),
        b2=np.zeros(64, np.float32),
        wl=(rng.standard_normal((128, 64)) * 0.1).astype(np.float32),
        bl=np.zeros(128, np.float32),
        num_samples=K,
    )
    print("kernel out:", kernel(**ins))


# revision 19
# speedup vs baseline: 1.7213x; 1.7213x over previous
"""Trainium2 Bass kernel for nn_AppearanceLoss (keypoint patch CNN MSE).

Host: crops 33x33 patches at keypoint locations (data-dependent indices),
builds the conv1 im2col directly (9 shifted copies + ones row for the bias),
shards 256 keypoints across 8 NeuronCores.
Device: conv1 = single K=109 matmul per 496-pixel chunk (im2col weights);
conv2 = offset-accumulated K=64 matmuls over patch pairs, two concurrent
row-tiles, N=450 (two quads per matmul); GAP via DVE reduce; linear on
feature diffs (bias cancels); Square+accum partial sums.
Host: sums 8 per-core partials into the scalar MSE.
"""

import sys

sys.path.insert(0, "/opt/trn_rl_repo")

from contextlib import ExitStack

import ml_dtypes
import numpy as np

import concourse.bass as bass  # noqa: F401
import concourse.tile as tile
from concourse import bacc, bass_utils, mybir

SIGMA = 16
PATCH = 33  # 2*SIGMA+1
HOUT = 31  # conv1 valid output: 33-3+1
COUT = 15  # conv2 stride-2 valid output: (31-3)//2+1
B, K, H = 4, 64, 256
NCORES = 8
NKP = B * K  # 256 keypoints total
KPC = NKP // NCORES  # 32 keypoints per core
NPATCH = KPC * B  # 128 patches per core per set
NQ = NPATCH // 4  # 32 quads per set
NQT = 2 * NQ  # 64 quads total per core
KIM = 109  # conv1 im2col contraction: 9 offsets * 12 (patch,chan) + ones row
BF16 = mybir.dt.bfloat16
F32 = mybir.dt.float32
NPBF16 = ml_dtypes.bfloat16

_CACHE: dict = {}


def _build_graph():
    nc = bacc.Bacc(
        "TRN2",
        target_bir_lowering=False,
        debug=False,
        enable_asserts=False,
        num_devices=NCORES,
    )
    # conv1 im2col input: row o*12+3j+c = offset o, patch-in-quad j, chan c;
    # row 108 = const 1.0 (bias); col = quad (0..63), then 31x31 out pixels
    xim_d = nc.dram_tensor(
        "xim", [KIM, NQT, HOUT, HOUT], BF16, kind="ExternalInput"
    ).ap()
    w1_d = nc.dram_tensor("w1", [KIM, 128], BF16, kind="ExternalInput").ap()
    w2_d = nc.dram_tensor("w2", [128, 9, 128], BF16, kind="ExternalInput").ap()
    b2_d = nc.dram_tensor("b2", [128, 1], F32, kind="ExternalInput").ap()
    wl_d = nc.dram_tensor("wl", [128, 128], BF16, kind="ExternalInput").ap()
    out_d = nc.dram_tensor("out", [128, 2], F32, kind="ExternalOutput").ap()

    RELU = mybir.ActivationFunctionType.Relu
    SQUARE = mybir.ActivationFunctionType.Square

    with ExitStack() as ctx:
        tc = ctx.enter_context(tile.TileContext(nc))
        const = ctx.enter_context(tc.tile_pool(name="const", bufs=1))
        xpool = ctx.enter_context(tc.tile_pool(name="x", bufs=4))
        hpool = ctx.enter_context(tc.tile_pool(name="h", bufs=4))
        gpool = ctx.enter_context(tc.tile_pool(name="g", bufs=1))
        spool = ctx.enter_context(tc.tile_pool(name="scr", bufs=4))
        pp1 = ctx.enter_context(tc.tile_pool(name="pp1", bufs=3, space="PSUM"))
        pp2 = ctx.enter_context(tc.tile_pool(name="pp2", bufs=5, space="PSUM"))

        w1_t = const.tile([KIM, 128], BF16)
        nc.sync.dma_start(w1_t[:], w1_d)
        w2_t = const.tile([128, 9, 128], BF16)
        nc.sync.dma_start(w2_t[:], w2_d)
        b2_t = const.tile([128, 1], F32)
        nc.sync.dma_start(b2_t[:], b2_d)
        wl_t = const.tile([128, 128], BF16)
        nc.sync.dma_start(wl_t[:], wl_d)

        # gap col 2q+jj; partition 64a+m = patch (q, 2*jj+a) channel m
        gap = gpool.tile([128, NQT * 2], F32)
        res = gpool.tile([128, 2], F32)

        # one fused pipeline at 2-quad granularity:
        # DMA -> conv1 MM -> relu evict -> conv2 MM chain -> relu -> GAP
        for G in range(NQT // 2):  # 32 groups of 2 quads
            xim = xpool.tile([KIM, 2, HOUT, HOUT], BF16, tag="xim")
            if G % 2 == 0:
                nc.sync.dma_start(xim[:], xim_d[:, 2 * G : 2 * G + 2])
            else:
                nc.gpsimd.dma_start(xim[:], xim_d[:, 2 * G : 2 * G + 2])
            h1 = hpool.tile([128, 2, HOUT, HOUT], BF16, tag="h1")
            for k in range(2):
                for ci, (r0, nr) in enumerate(((0, 16), (16, 15))):
                    ps1 = pp1.tile([128, 16, HOUT], F32, tag="ps1")
                    nc.tensor.matmul(
                        ps1[:, :nr, :],
                        w1_t[:],
                        xim[:, k, r0 : r0 + nr, :],
                        start=True,
                        stop=True,
                    )
                    # relu evict (bias folded into im2col ones row);
                    # alternate engines to split the eviction load
                    if (k + ci) % 2 == 0:
                        nc.scalar.activation(
                            h1[:, k, r0 : r0 + nr, :], ps1[:, :nr, :], RELU
                        )
                    else:
                        nc.vector.tensor_scalar_max(
                            h1[:, k, r0 : r0 + nr, :], ps1[:, :nr, :], 0.0
                        )
            # conv2: N=450 (both quads), pairs in concurrent row-tiles
            for jj in range(2):
                p0 = 64 * jj
                ps2 = pp2.tile(
                    [128, 2, COUT * COUT], F32, tag="ps2", name=f"ps2_{jj}"
                )
                for o in range(9):
                    dy, dx = o // 3, o % 3
                    nc.tensor.matmul(
                        ps2[:],
                        w2_t[p0 : p0 + 64, o, :],
                        h1[p0 : p0 + 64, :, dy : dy + 29 : 2, dx : dx + 29 : 2],
                        start=(o == 0),
                        stop=(o == 8),
                        tile_position=(p0, 0),
                    )
                scr = spool.tile([128, 2, COUT * COUT], F32, tag="scr")
                nc.scalar.activation(
                    scr[:],
                    ps2[:],
                    RELU,
                    bias=b2_t[:],
                    scale=1.0 / (COUT * COUT),
                )
                # GAP: per-quad sums -> gap cols 2*(2G+k)+jj, k=0,1
                c0 = 2 * (2 * G) + jj
                nc.vector.tensor_reduce(
                    gap[:, c0 : c0 + 3 : 2],
                    scr[:],
                    axis=mybir.AxisListType.X,
                    op=mybir.AluOpType.add,
                )

        # linear on feature diffs (linear bias cancels), squared sums
        dg = spool.tile([128, NQ * 2], F32, tag="dg")
        nc.vector.tensor_sub(dg[:], gap[:, 0 : NQ * 2], gap[:, NQ * 2 : NQT * 2])
        dgb = spool.tile([128, NQ * 2], BF16, tag="dgb")
        nc.vector.tensor_copy(dgb[:], dg[:])
        for jj in range(2):
            p0 = 64 * jj
            ps3 = pp2.tile([128, NQ * 2], F32, tag="ps2", name=f"ps3_{jj}")
            nc.tensor.matmul(
                ps3[:],
                wl_t[p0 : p0 + 64, :],
                dgb[p0 : p0 + 64, :],
                start=True,
                stop=True,
                tile_position=(p0, 0),
            )
            scr3 = spool.tile([128, NQ * 2], F32, tag="scr3", name=f"scr3_{jj}")
            nc.scalar.activation(
                scr3[:], ps3[:], SQUARE, accum_out=res[:, jj : jj + 1]
            )
        nc.sync.dma_start(out_d, res[:])

    nc.compile()
    return nc


def _prep_weights(w1, b1, w2, b2, wl):
    w1im = np.zeros((KIM, 128), np.float32)
    for j in range(4):
        for c in range(3):
            for o in range(9):
                dy, dx = o // 3, o % 3
                w1im[12 * o + 3 * j + c, 32 * j : 32 * j + 32] = w1[:, c, dy, dx]
    w1im[108, :] = np.tile(b1, 4)
    w2blk = np.zeros((128, 9, 128), np.float32)
    for jj in range(2):
        for j in range(2):
            for c in range(32):
                for o in range(9):
                    dy, dx = o // 3, o % 3
                    w2blk[64 * jj + 32 * j + c, o, 64 * j : 64 * j + 64] = w2[
                        :, c, dy, dx
                    ]
    b2q = np.tile(b2 / (COUT * COUT), 2)[:, None].astype(np.float32)
    wlrep = np.zeros((128, 128), np.float32)
    wlrep[0:64] = wl.T
    wlrep[64:128] = wl.T
    return (
        w1im.astype(NPBF16),
        w2blk.astype(NPBF16),
        np.ascontiguousarray(b2q),
        wlrep.astype(NPBF16),
    )


def _crop_all(images, kps):
    # images [B,3,H,W] f32; kps [NKP,2] normalized -> patches [NKP,B,3,P,P]
    hw = images.shape[-1]
    px = kps.astype(np.float32) * np.float32(hw)
    starts = np.clip(np.floor(px).astype(np.int32) - SIGMA, 0, hw - PATCH)
    out = np.empty((kps.shape[0], images.shape[0], 3, PATCH, PATCH), np.float32)
    for n in range(kps.shape[0]):
        x, y = int(starts[n, 0]), int(starts[n, 1])
        out[n] = images[:, :, y : y + PATCH, x : x + PATCH]
    return out


def _im2col(pat):
    # [128,3,33,33] -> [KIM, 32, 31, 31]: row 12o+3j+c = patch data shifted
    # by offset o's (dy,dx); row 108 = 1.0
    pat4 = pat.reshape(NQ, 4, 3, PATCH, PATCH)
    out = np.empty((KIM, NQ, HOUT, HOUT), np.float32)
    for o in range(9):
        dy, dx = o // 3, o % 3
        sh = pat4[:, :, :, dy : dy + HOUT, dx : dx + HOUT]
        out[12 * o : 12 * o + 12] = sh.transpose(1, 2, 0, 3, 4).reshape(
            12, NQ, HOUT, HOUT
        )
    out[108] = 1.0
    return out


def _make_in_maps(np_inputs):
    images_ground = np.asarray(np_inputs["images_ground"], np.float32)
    images_satellite = np.asarray(np_inputs["images_satellite"], np.float32)
    kg = np.asarray(np_inputs["keypoints_ground"], np.float32).reshape(-1, 2)
    ks = np.asarray(np_inputs["keypoints_satellite"], np.float32).reshape(-1, 2)
    w1 = np.asarray(np_inputs["w1"], np.float32)
    b1 = np.asarray(np_inputs["b1"], np.float32)
    w2 = np.asarray(np_inputs["w2"], np.float32)
    b2 = np.asarray(np_inputs["b2"], np.float32)
    wl = np.asarray(np_inputs["wl"], np.float32)

    pg = _crop_all(images_ground, kg)  # [256,4,3,33,33]
    ps = _crop_all(images_satellite, ks)
    w1im, w2blk, b2q, wlrep = _prep_weights(w1, b1, w2, b2, wl)

    in_maps = []
    for i in range(NCORES):
        sl = slice(i * KPC, (i + 1) * KPC)
        patg = pg[sl].reshape(NPATCH, 3, PATCH, PATCH)
        pats = ps[sl].reshape(NPATCH, 3, PATCH, PATCH)
        xim = np.concatenate([_im2col(patg), _im2col(pats)], axis=1).astype(NPBF16)
        in_maps.append(dict(xim=xim, w1=w1im, w2=w2blk, b2=b2q, wl=wlrep))
    return in_maps


def kernel(**inputs):
    in_maps = _make_in_maps(inputs)

    if "nc" not in _CACHE:
        _CACHE["nc"] = _build_graph()
    nc = _CACHE["nc"]

    results = bass_utils.run_bass_kernel_spmd(
        nc, in_maps, core_ids=list(range(NCORES))
    )
    total = np.float64(0.0)
    for r in results.results:
        total += np.asarray(r["out"], np.float64).sum()
    mse = total / (NKP * B * 128)
    return np.asarray(mse, np.float32)


if __name__ == "__main__":
    rng = np.random.default_rng(0)
    ins = dict(
        images_ground=rng.standard_normal((B, 3, H, H)).astype(np.float32),
        images_satellite=rng.standard_normal((B, 3, H, H)).astype(np.float32),
        keypoints_ground=(0.2 + 0.6 * rng.random((B, K, 2))).astype(np.float32),
        keypoints_satellite=(0.2 + 0.6 * rng.random((B, K, 2))).astype(np.float32),
        w1=(rng.standard_normal((32, 3, 3, 3)) * 0.1).astype(np.float32),
        b1=np.zeros(32, np.float32),
        w2=(rng.standard_normal((64, 32, 3, 3)) * 0.05).astype(np.float32),
        b2=np.zeros(64, np.float32),
        wl=(rng.standard_normal((128, 64)) * 0.1).astype(np.float32),
        bl=np.zeros(128, np.float32),
        num_samples=K,
    )
    print("kernel out:", kernel(**ins))
